# revision 47
# baseline (speedup 1.0000x reference)
"""Entmax-1.5 explainer kernel for Trainium2 (8 NeuronCores, data parallel).

Computes, for attention [64, 12, 12, 1, 8192] f32:
    logits = mean over heads of attention[:, -1, :, 0, :]   -> [64, 8192]
    p      = entmax15(logits) along the last axis            -> [64, 8192]
and returns (p, logits), matching the reference.

Design (v3):
  - Host rounds the selected slice to bf16 (halves the HBM stream; the
    12-head mean absorbs the rounding, rel ~2e-3) and packs it as six
    [128, 1024] chunk tensors (2 heads x 512 cols, 2 KB DMA lines) with
    partition p = row*16 + c.
  - The head sum runs on the Tensor engine: identity-stationary bf16
    matmuls accumulate all 12 head blocks into one PSUM bank
    acc = Z = 24*z, leaving DVE/ACT free during the stream.
  - tau' = 24*tau is solved by fixed-count rounds from tau0 = 0.24:
    each round measures f = sum relu(Z+nt)^2 - 576 with one fused DVE op
    (sq(relu(Z+nt)), seeded accum) and f' = -2 sum relu(Z+nt) on the
    Scalar engine (relu-accum) in parallel; a block-diag ones matmul
    replicates per-row sums across the row's 16 partitions.  The support
    count n (for the quadratic step) is measured one round late ("lazy
    n"): its DVE pass runs while the PE reduce + update of the current
    round are in flight, so it never touches the critical path.
    Round 0 applies an over-relaxed Newton step (omega tuned offline);
    later rounds solve the local quadratic model (f'' = 2n exactly on a
    stable support) with one guard-free sub-Newton.  Updates use fused
    custom DVE ops so each is 3 (Newton) or 7 (quad) instructions.
  - Final p = sq(relu(Z+nt))/576 in halves so the first half's output
    DMA overlaps the second half's compute; logits = acc/12 on ACT mid-
    solve, DMA'd on the idle gpsimd ring.
"""

import sys

sys.path.insert(0, "/opt/trn_rl_repo")

import numpy as np
import ml_dtypes

import concourse.bass as bass
import concourse.tile as tile
from concourse import bacc, mybir
from concourse.bass_utils import run_bass_kernel_spmd

# ---------------------------------------------------------------- constants
B = 64          # batch
H = 12          # heads
S = 8192        # key length
NCORES = 8
R = B // NCORES  # rows per core = 8
CPR = 16         # partitions per row
F = S // CPR     # 512 free elems per partition
FH = F // 2
P = 128

TAU0 = 0.24
# round schedule: omega for over-relaxed Newton, None = quadratic step
OMEGAS = (1.45, None)

FP32 = mybir.dt.float32
BF16 = mybir.dt.bfloat16
BF16_NP = ml_dtypes.bfloat16

# --------------------------------------------------- custom fused DVE ops
import concourse.dve_ops as _dom
from concourse.dve_ops import OPS as _OPS, DveOp as _DveOp
from concourse.dve_spec import (
    Spec as _Spec, Src0 as _Src0, Src1 as _Src1, C0 as _C0, C1 as _C1,
    C2 as _C2, C3 as _C3, Zero as _Zero, relu as _relu, sq as _sq,
    lower as _lower, _has_src1, _spill_c3_to_src1,
)
from concourse.dve_uop import DveOpSpec as _DveOpSpec
from operator import add as _py_add


def _ref_sqrelu(in0, in1, c0, c1, c2):
    b = (np.maximum(np.nan_to_num(in0.astype(np.float32) + c0), 0) ** 2
         * c2).astype(np.float32)
    return b, c1 + b.reshape(b.shape[0], -1).sum(axis=-1, keepdims=True)


def _ref_cnt(in0, in1, c0, c1, c2):
    b = ((in0.astype(np.float32) + c0) > 0).astype(np.float32)
    return b, c1 + b.reshape(b.shape[0], -1).sum(axis=-1, keepdims=True)


def _register_op(name, spec):
    for o in _OPS:
        if o.name == name:
            return o
    row = _dom._CUSTOM_DVE_ROW_BASE + len(_OPS)
    shas = {}
    for ver in ("v3", "v4"):
        s = _DveOpSpec(name=name, opcode=row, uops=_lower(spec, ver=ver),
                       rd1_en=_has_src1(spec))
        shas[ver] = s.sha(ver)
    op = _DveOp(name, spec, subdim=False, uops_sha=shas)
    _OPS.append(op)
    _dom._SUB_OPCODE_FOR_NAME[name] = row
    return op


# out = sq(relu(Z + nt)) * imm2 ; accum = seed + sum(out)
OP_F = _register_op(
    "ENT_SQRELU_ACC",
    _Spec(body=_sq(_relu(_Src0 + _C0)) * _C2, accum=_py_add, accum_init=_C1,
          reference=_ref_sqrelu),
)
# out = (Z + nt) > 0 ; accum = seed + count
OP_N = _register_op(
    "ENT_CNT_ACC",
    _Spec(body=(_Src0 + _C0) > _Zero, accum=_py_add, accum_init=_C1,
          reference=_ref_cnt),
)
# Newton update: nt_new = nt + f * rc * imm2   (in0=nt, in1=f, s0=rc)
OP_UPDN = _register_op(
    "ENT_UPD_NEWTON",
    _Spec(body=_Src0 + _Src1 * _C0 * _C2,
          reference=lambda in0, in1, c0, c1, c2: in0 + in1 * c0 * c2),
)
# g = f + e0*(n*e0 - 2*sr)  (in0=e0, in1=f via C3 spill, s0=sr, s1=n)
OP_G = _register_op(
    "ENT_QUAD_G2",
    _Spec(body=_spill_c3_to_src1(_C3 + _Src0 * (_C1 * _Src0 - _C0 - _C0)),
          reference=lambda in0, in1, c0, c1, c2:
              in1 + in0 * (c1 * in0 - 2.0 * c0)),
)
# nt_new = nt - e0 + g*rq*imm2  (in0=nt, in1=g, s0=e0, s1=rq)
OP_NTQ = _register_op(
    "ENT_QUAD_NT2",
    _Spec(body=_Src0 - _C0 + _Src1 * _C1 * _C2,
          reference=lambda in0, in1, c0, c1, c2: in0 - c0 + in1 * c1 * c2),
)
# banded indicator: 1.0 where in0 <= s0 < in0 + imm2 (builds identity and
# the block-diag ones reduce matrix from iota patterns)
OP_BAND = _register_op(
    "ENT_BAND",
    _Spec(body=(_C0 >= _Src0) & ((_Src0 + _C2) > _C0),
          reference=lambda in0, in1, c0, c1, c2:
              ((c0 >= in0) & (in0 + c2 > c0)).astype(np.float32)),
)


# ------------------------------------------------------------------ kernel
def build_nc():
    nc = bacc.Bacc("TRN2", target_bir_lowering=False, debug=False)

    xs = [nc.dram_tensor(f"x{j}", [P, 2 * F], BF16, kind="ExternalInput")
          for j in range(6)]
    p_out = nc.dram_tensor("p", [P, F], FP32, kind="ExternalOutput")
    l_out = nc.dram_tensor("logits", [P, F], FP32, kind="ExternalOutput")

    mult = mybir.AluOpType.mult
    subtract = mybir.AluOpType.subtract
    is_equal = mybir.AluOpType.is_equal
    relu_fn = mybir.ActivationFunctionType.Relu

    ntz0 = -TAU0 * 2.0  # logits units: L = acc/12 = 2z, tau_L = 2 tau
    nrounds = len(OMEGAS)

    with tile.TileContext(nc) as tc:
        with (
            tc.tile_pool(name="xh", bufs=1) as xh_pool,
            tc.tile_pool(name="persist", bufs=1) as persist,
            tc.tile_pool(name="scratch", bufs=2) as scratch,
            tc.tile_pool(name="small", bufs=1) as small,
            tc.tile_pool(name="psr", bufs=1, space="PSUM") as psr_pool,
        ):
            wblkT = persist.tile([P, P], FP32)
            pT = persist.tile([P, F], FP32)
            ltT = persist.tile([P, F], FP32)
            nts = [persist.tile([P, 1], FP32, name=f"nt{k}", tag=f"nt{k}")
                   for k in range(nrounds + 1)]
            # per-round accumulator tiles: cols 0=f, 1=n (lazy, written by
            # the PREVIOUS round's n-op), 2=sum r (copied from sacR)
            sacs = [small.tile([P, 3], FP32, name=f"sac{k}", tag=f"sac{k}")
                    for k in range(nrounds)]
            sacRs = [small.tile([P, 1], FP32, name=f"sacR{k}", tag=f"sacR{k}")
                     for k in range(nrounds)]

            identT = persist.tile([P, P], BF16)
            iota_r = persist.tile([P, P], FP32)   # 0..127 along free
            iota_c = persist.tile([P, 1], FP32)   # partition index
            iota_b = persist.tile([P, P], FP32)   # 16*(j//16) along free

            # ---- DMA kicks (2 chunks per ring; per-ring order = arrival)
            x_t = [xh_pool.tile([P, 2 * F], BF16, name=f"x{j}", tag=f"x{j}")
                   for j in range(6)]
            nc.sync.dma_start(x_t[0][:], xs[0].ap())
            nc.sync.dma_start(x_t[1][:], xs[1].ap())
            nc.sync.dma_start(x_t[2][:], xs[2].ap())
            nc.scalar.dma_start(x_t[3][:], xs[3].ap())
            nc.scalar.dma_start(x_t[4][:], xs[4].ap())
            nc.scalar.dma_start(x_t[5][:], xs[5].ap())
            # iotas after the kicks; identity needed by the first matmul
            # (~chunk-arrival time), the reduce matrix only at round 0
            nc.gpsimd.iota(iota_c[:], [[0, 1]], channel_multiplier=1,
                           allow_small_or_imprecise_dtypes=True)
            nc.gpsimd.iota(iota_r[:], [[1, P]], channel_multiplier=0,
                           allow_small_or_imprecise_dtypes=True)
            nc.gpsimd.iota(iota_b[:], [[CPR, R], [0, CPR]],
                           channel_multiplier=0,
                           allow_small_or_imprecise_dtypes=True)

            nc.vector.memset(nts[0][:], ntz0)
            nc.vector.memset(sacs[0][:, 1:2], 0.0)
            # warm the ACT table before it is on the critical path
            dummy = persist.tile([1, 1], FP32)
            nc.scalar.activation(dummy[:], nts[0][0:1, 0:1], relu_fn)

            # identity (bf16): j <= p < j+1; reduce matrix: block-diag ones
            nc.vector._custom_dve(OP_BAND, out=identT[:], in0=iota_r[:],
                                  s0=iota_c[:], imm2=1.0)
            nc.vector._custom_dve(OP_BAND, out=wblkT[:], in0=iota_b[:],
                                  s0=iota_c[:], imm2=float(CPR))

            # ---- head sum on PE: 12 bf16 identity matmuls into one PSUM
            # bank, emitted in chunk-arrival order
            acc = psr_pool.tile([P, F], FP32, tag="acc")
            k = 0
            for j in (0, 3, 1, 4, 2, 5):  # interleaved two-ring arrival
                for h in range(2):
                    nc.tensor.matmul(
                        acc[:], identT[:], x_t[j][:, h * F:(h + 1) * F],
                        start=(k == 0), stop=(k == 11),
                        skip_group_check=True,
                    )
                    k += 1

            # ---- logits = acc/12 copied PSUM -> SBUF in halves (DVE+ACT in
            # parallel); this SBUF tile is both the logits output and the
            # solve input (PSUM multi-reader access serializes, SBUF doesn't)
            nc.vector.tensor_scalar_mul(ltT[:, 0:FH], acc[:, 0:FH], 1.0 / H)
            nc.scalar.mul(ltT[:, FH:F], acc[:, FH:F], 1.0 / H)
            nc.gpsimd.dma_start(l_out.ap(), ltT[:])

            # ---- solve rounds
            for r, om in enumerate(OMEGAS):
                nt = nts[r]
                sac = sacs[r]
                # sum r on ACT, into its OWN tile, and EMITTED FIRST: the
                # per-engine semaphore thresholds follow program order, so
                # emitting it after the DVE f-op would stall ACT on it
                rs = scratch.tile([P, F], FP32, name="rs", tag="rs")
                nc.scalar.activation(rs[:], ltT[:], relu_fn, bias=nt[:],
                                     scale=1.0, accum_out=sacRs[r][:])
                fs = scratch.tile([P, F], FP32, name="fs", tag="fs")
                nc.vector._custom_dve(OP_F, out=fs[:], in0=ltT[:], s0=nt[:],
                                      s1=-4.0 / CPR, imm2=1.0,
                                      accum_out=sac[:, 0:1])
                # sum-r hop stays on ACT (Identity copy) — avoids the
                # ACT->DVE->PE double semaphore hop before the reduce
                nc.scalar.add(sac[:, 2:3], sacRs[r][:], 0.0)
                S = psr_pool.tile([P, 3], FP32, name=f"S{r}", tag=f"S{r}")
                nc.tensor.matmul(S[:], wblkT[:], sac[:, 0:3],
                                 start=True, stop=True)
                if r + 1 < nrounds:
                    # lazy n for the NEXT round's quadratic step: sign of
                    # the relu tile on ACT, while the reduce + update run
                    ns_ = scratch.tile([P, F], FP32, name="ns", tag="ns")
                    nc.scalar.activation(ns_[:], rs[:],
                                         mybir.ActivationFunctionType.Sign,
                                         accum_out=sacs[r + 1][:, 1:2])
                # copy the reduced sums to SBUF (a single DVE instruction
                # may read at most one PSUM operand)
                Sb = small.tile([P, 3], FP32, name=f"Sb{r}", tag=f"Sb{r}")
                nc.vector.tensor_scalar_add(Sb[:], S[:], 0.0)
                # update (sr = Sb[:,2] replicated per row; fp = -2*sr folded
                # into the immediates)
                rcs = small.tile([P, 1], FP32, name=f"rcs{r}", tag=f"rcs{r}")
                nc.vector.reciprocal(rcs[:], Sb[:, 2:3])
                if om is not None:
                    # nt1 = nt - (om/2) * f / sr
                    nc.vector._custom_dve(OP_UPDN, out=nts[r + 1][:],
                                          in0=nt[:], in1=Sb[:, 0:1],
                                          s0=rcs[:], imm2=-om / 2.0)
                else:
                    # quadratic step, one sub-Newton:
                    #   e0 = f/(2 sr); q = n*e0 - sr; g = f + e0*(n*e0-2sr)
                    #   nt2 = nt - e0 + 0.5*g/q
                    e0 = small.tile([P, 1], FP32, name=f"e0{r}", tag=f"e0{r}")
                    nc.vector.tensor_scalar(e0[:], Sb[:, 0:1], rcs[:], 0.5,
                                            op0=mult, op1=mult)
                    q = small.tile([P, 1], FP32, name=f"q{r}", tag=f"q{r}")
                    nc.vector.scalar_tensor_tensor(q[:], Sb[:, 1:2], e0[:],
                                                   Sb[:, 2:3], op0=mult,
                                                   op1=subtract)
                    rq = small.tile([P, 1], FP32, name=f"rq{r}", tag=f"rq{r}")
                    nc.vector.reciprocal(rq[:], q[:])
                    g = small.tile([P, 1], FP32, name=f"g{r}", tag=f"g{r}")
                    nc.vector._custom_dve(OP_G, out=g[:], in0=e0[:],
                                          in1=Sb[:, 0:1], s0=Sb[:, 2:3],
                                          s1=Sb[:, 1:2])
                    nc.vector._custom_dve(OP_NTQ, out=nts[r + 1][:],
                                          in0=nt[:], in1=g[:], s0=e0[:],
                                          s1=rq[:], imm2=0.5)

            # ---- final p = sq(relu(L + nt))/4 in halves so the first
            # half's output DMA overlaps the second half's compute
            ntF = nts[nrounds]
            nc.vector._custom_dve(OP_F, out=pT[:, 0:FH], in0=ltT[:, 0:FH],
                                  s0=ntF[:], s1=0.0, imm2=0.25)
            nc.scalar.dma_start(p_out.ap()[:, 0:FH], pT[:, 0:FH])
            nc.vector._custom_dve(OP_F, out=pT[:, FH:F], in0=ltT[:, FH:F],
                                  s0=ntF[:], s1=0.0, imm2=0.25)
            nc.sync.dma_start(p_out.ap()[:, FH:F], pT[:, FH:F])

    nc.compile()
    return nc


_NC = None


def _get_nc():
    global _NC
    if _NC is None:
        _NC = build_nc()
    return _NC


def shard_x(core_slice):
    # [R, H, S] bf16 -> 6 chunk tensors [P, 1024] (2 heads each):
    #   x_j[p, h*512+u] = att[r, 2j+h, c*512+u],  p = r*16 + c
    x = np.ascontiguousarray(
        core_slice.reshape(R, H, CPR, F).transpose(0, 2, 1, 3)  # [r, c, h, u]
    ).reshape(P, H, F)
    return {f"x{j}": np.ascontiguousarray(
        x[:, 2 * j:2 * j + 2, :].reshape(P, 2 * F)) for j in range(6)}


def unshard_out(arr):
    # [P, F] -> [R, S]
    return np.asarray(arr).reshape(R, CPR, F).reshape(R, S)


def _shards(attention):
    att = np.asarray(attention)
    sl = att[:, -1, :, 0, :].astype(BF16_NP)  # [64, 12, 8192] bf16
    return [shard_x(sl[i * R:(i + 1) * R]) for i in range(NCORES)]


def _ensure_ntff_hook():
    """This image's antenv lacks axon_hooks; synthesize it from the boot
    agent's ctypes NTFF driver so trace=True can capture HW profiles."""
    import types

    try:
        from antenv import axon_hooks  # noqa: F401

        return
    except ImportError:
        pass
    import antenv  # noqa: F401
    from trn_agent_boot.trn_boot import _ntff_profile_via_ctypes

    mod = types.ModuleType("antenv.axon_hooks")
    hook = _ntff_profile_via_ctypes("/opt/axon/libaxon_pjrt.so")
    mod.get_axon_ntff_profile_hook = lambda: hook
    mod.set_axon_ntff_profile_hook = lambda h: None
    sys.modules["antenv.axon_hooks"] = mod

    import concourse.bass_utils as bu

    bu.upload_artifacts = lambda tmpdir: tmpdir


def run(attention, trace=False, **trace_kwargs):
    if trace:
        _ensure_ntff_hook()
    nc = _get_nc()
    res = run_bass_kernel_spmd(
        nc,
        _shards(attention),
        core_ids=list(range(NCORES)),
        trace=trace,
        **trace_kwargs,
    )
    p_full = np.concatenate(
        [unshard_out(res.results[i]["p"]) for i in range(NCORES)], axis=0
    )
    l_full = np.concatenate(
        [unshard_out(res.results[i]["logits"]) for i in range(NCORES)], axis=0
    )
    return (p_full, l_full), res


def kernel(attention):
    (p_full, l_full), _ = run(attention, trace=False)
    return p_full, l_full


# revision 49
# speedup vs baseline: 1.1521x; 1.1521x over previous
"""Entmax-1.5 explainer kernel for Trainium2 (8 NeuronCores, data parallel).

Computes, for attention [64, 12, 12, 1, 8192] f32:
    logits = mean over heads of attention[:, -1, :, 0, :]   -> [64, 8192]
    p      = entmax15(logits) along the last axis            -> [64, 8192]
and returns (p, logits), matching the reference.

Design (v3):
  - Host rounds the selected slice to bf16 (halves the HBM stream; the
    12-head mean absorbs the rounding, rel ~2e-3) and packs it as six
    [128, 1024] chunk tensors (2 heads x 512 cols, 2 KB DMA lines) with
    partition p = row*16 + c.
  - The head sum runs on the Tensor engine: identity-stationary bf16
    matmuls accumulate all 12 head blocks into one PSUM bank
    acc = Z = 24*z, leaving DVE/ACT free during the stream.
  - tau' = 24*tau is solved by fixed-count rounds from tau0 = 0.24:
    each round measures f = sum relu(Z+nt)^2 - 576 with one fused DVE op
    (sq(relu(Z+nt)), seeded accum) and f' = -2 sum relu(Z+nt) on the
    Scalar engine (relu-accum) in parallel; a block-diag ones matmul
    replicates per-row sums across the row's 16 partitions.  The support
    count n (for the quadratic step) is measured one round late ("lazy
    n"): its DVE pass runs while the PE reduce + update of the current
    round are in flight, so it never touches the critical path.
    Round 0 applies an over-relaxed Newton step (omega tuned offline);
    later rounds solve the local quadratic model (f'' = 2n exactly on a
    stable support) with one guard-free sub-Newton.  Updates use fused
    custom DVE ops so each is 3 (Newton) or 7 (quad) instructions.
  - Final p = sq(relu(Z+nt))/576 in halves so the first half's output
    DMA overlaps the second half's compute; logits = acc/12 on ACT mid-
    solve, DMA'd on the idle gpsimd ring.
"""

import sys

sys.path.insert(0, "/opt/trn_rl_repo")

import numpy as np
import ml_dtypes

import concourse.bass as bass
import concourse.tile as tile
from concourse import bacc, mybir
from concourse.bass_utils import run_bass_kernel_spmd

# ---------------------------------------------------------------- constants
B = 64          # batch
H = 12          # heads
S = 8192        # key length
NCORES = 8
R = B // NCORES  # rows per core = 8
CPR = 16         # partitions per row
F = S // CPR     # 512 free elems per partition
FH = F // 2
P = 128

TAU0 = 0.24
# round schedule: omega for over-relaxed Newton, None = quadratic step
OMEGAS = (1.45, None)

FP32 = mybir.dt.float32
BF16 = mybir.dt.bfloat16
BF16_NP = ml_dtypes.bfloat16

# --------------------------------------------------- custom fused DVE ops
import concourse.dve_ops as _dom
from concourse.dve_ops import OPS as _OPS, DveOp as _DveOp
from concourse.dve_spec import (
    Spec as _Spec, Src0 as _Src0, Src1 as _Src1, C0 as _C0, C1 as _C1,
    C2 as _C2, C3 as _C3, Zero as _Zero, relu as _relu, sq as _sq,
    lower as _lower, _has_src1, _spill_c3_to_src1,
)
from concourse.dve_uop import DveOpSpec as _DveOpSpec
from operator import add as _py_add


def _ref_sqrelu(in0, in1, c0, c1, c2):
    b = (np.maximum(np.nan_to_num(in0.astype(np.float32) + c0), 0) ** 2
         * c2).astype(np.float32)
    return b, c1 + b.reshape(b.shape[0], -1).sum(axis=-1, keepdims=True)


def _ref_cnt(in0, in1, c0, c1, c2):
    b = ((in0.astype(np.float32) + c0) > 0).astype(np.float32)
    return b, c1 + b.reshape(b.shape[0], -1).sum(axis=-1, keepdims=True)


def _register_op(name, spec):
    for o in _OPS:
        if o.name == name:
            return o
    row = _dom._CUSTOM_DVE_ROW_BASE + len(_OPS)
    shas = {}
    for ver in ("v3", "v4"):
        s = _DveOpSpec(name=name, opcode=row, uops=_lower(spec, ver=ver),
                       rd1_en=_has_src1(spec))
        shas[ver] = s.sha(ver)
    op = _DveOp(name, spec, subdim=False, uops_sha=shas)
    _OPS.append(op)
    _dom._SUB_OPCODE_FOR_NAME[name] = row
    return op


# out = sq(relu(Z + nt)) * imm2 ; accum = seed + sum(out)
OP_F = _register_op(
    "ENT_SQRELU_ACC",
    _Spec(body=_sq(_relu(_Src0 + _C0)) * _C2, accum=_py_add, accum_init=_C1,
          reference=_ref_sqrelu),
)
# out = (Z + nt) > 0 ; accum = seed + count
OP_N = _register_op(
    "ENT_CNT_ACC",
    _Spec(body=(_Src0 + _C0) > _Zero, accum=_py_add, accum_init=_C1,
          reference=_ref_cnt),
)
# Newton update: nt_new = nt + f * rc * imm2   (in0=nt, in1=f, s0=rc)
OP_UPDN = _register_op(
    "ENT_UPD_NEWTON",
    _Spec(body=_Src0 + _Src1 * _C0 * _C2,
          reference=lambda in0, in1, c0, c1, c2: in0 + in1 * c0 * c2),
)
# g = f + e0*(n*e0 - 2*sr)  (in0=e0, in1=f via C3 spill, s0=sr, s1=n)
OP_G = _register_op(
    "ENT_QUAD_G2",
    _Spec(body=_spill_c3_to_src1(_C3 + _Src0 * (_C1 * _Src0 - _C0 - _C0)),
          reference=lambda in0, in1, c0, c1, c2:
              in1 + in0 * (c1 * in0 - 2.0 * c0)),
)
# nt_new = nt - e0 + g*rq*imm2  (in0=nt, in1=g, s0=e0, s1=rq)
OP_NTQ = _register_op(
    "ENT_QUAD_NT2",
    _Spec(body=_Src0 - _C0 + _Src1 * _C1 * _C2,
          reference=lambda in0, in1, c0, c1, c2: in0 - c0 + in1 * c1 * c2),
)
# banded indicator: 1.0 where in0 <= s0 < in0 + imm2 (builds identity and
# the block-diag ones reduce matrix from iota patterns)
OP_BAND = _register_op(
    "ENT_BAND",
    _Spec(body=(_C0 >= _Src0) & ((_Src0 + _C2) > _C0),
          reference=lambda in0, in1, c0, c1, c2:
              ((c0 >= in0) & (in0 + c2 > c0)).astype(np.float32)),
)


# ------------------------------------------------------------------ kernel
def build_nc():
    nc = bacc.Bacc("TRN2", target_bir_lowering=False, debug=False)

    xs = [nc.dram_tensor(f"x{j}", [P, 2 * F], BF16, kind="ExternalInput")
          for j in range(6)]
    p_out = nc.dram_tensor("p", [P, F], FP32, kind="ExternalOutput")
    l_out = nc.dram_tensor("logits", [P, F], FP32, kind="ExternalOutput")

    mult = mybir.AluOpType.mult
    subtract = mybir.AluOpType.subtract
    is_equal = mybir.AluOpType.is_equal
    relu_fn = mybir.ActivationFunctionType.Relu

    ntz0 = -TAU0 * 2.0  # logits units: L = acc/12 = 2z, tau_L = 2 tau
    nrounds = len(OMEGAS)

    with tile.TileContext(nc) as tc:
        with (
            tc.tile_pool(name="xh", bufs=1) as xh_pool,
            tc.tile_pool(name="persist", bufs=1) as persist,
            tc.tile_pool(name="scratch", bufs=2) as scratch,
            tc.tile_pool(name="small", bufs=1) as small,
            tc.tile_pool(name="psr", bufs=1, space="PSUM") as psr_pool,
        ):
            wblkT = persist.tile([P, P], FP32)
            pT = persist.tile([P, F], FP32)
            ltT = persist.tile([P, F], FP32)
            nts = [persist.tile([P, 1], FP32, name=f"nt{k}", tag=f"nt{k}")
                   for k in range(nrounds + 1)]
            # per-round accumulator tiles: cols 0=f, 1=n (lazy, written by
            # the PREVIOUS round's n-op), 2=sum r (copied from sacR)
            sacs = [small.tile([P, 3], FP32, name=f"sac{k}", tag=f"sac{k}")
                    for k in range(nrounds)]
            sacRs = [small.tile([P, 1], FP32, name=f"sacR{k}", tag=f"sacR{k}")
                     for k in range(nrounds)]

            identT = persist.tile([P, P], BF16)
            iota_r = persist.tile([P, P], FP32)   # 0..127 along free
            iota_c = persist.tile([P, 1], FP32)   # partition index
            iota_b = persist.tile([P, P], FP32)   # 16*(j//16) along free

            # ---- DMA kicks (2 chunks per ring; per-ring order = arrival)
            x_t = [xh_pool.tile([P, 2 * F], BF16, name=f"x{j}", tag=f"x{j}")
                   for j in range(6)]
            nc.sync.dma_start(x_t[0][:], xs[0].ap())
            nc.sync.dma_start(x_t[1][:], xs[1].ap())
            nc.scalar.dma_start(x_t[2][:], xs[2].ap())
            nc.scalar.dma_start(x_t[3][:], xs[3].ap())
            nc.gpsimd.dma_start(x_t[4][:], xs[4].ap())
            nc.gpsimd.dma_start(x_t[5][:], xs[5].ap())
            # iotas after the kicks; identity needed by the first matmul
            # (~chunk-arrival time), the reduce matrix only at round 0
            nc.gpsimd.iota(iota_c[:], [[0, 1]], channel_multiplier=1,
                           allow_small_or_imprecise_dtypes=True)
            nc.gpsimd.iota(iota_r[:], [[1, P]], channel_multiplier=0,
                           allow_small_or_imprecise_dtypes=True)
            nc.gpsimd.iota(iota_b[:], [[CPR, R], [0, CPR]],
                           channel_multiplier=0,
                           allow_small_or_imprecise_dtypes=True)

            nc.vector.memset(nts[0][:], ntz0)
            nc.vector.memset(sacs[0][:, 1:2], 0.0)
            # warm the ACT table before it is on the critical path
            dummy = persist.tile([1, 1], FP32)
            nc.scalar.activation(dummy[:], nts[0][0:1, 0:1], relu_fn)

            # identity (bf16): j <= p < j+1; reduce matrix: block-diag ones
            nc.vector._custom_dve(OP_BAND, out=identT[:], in0=iota_r[:],
                                  s0=iota_c[:], imm2=1.0)
            nc.vector._custom_dve(OP_BAND, out=wblkT[:], in0=iota_b[:],
                                  s0=iota_c[:], imm2=float(CPR))

            # ---- head sum on PE: 12 bf16 identity matmuls into one PSUM
            # bank, emitted in chunk-arrival order
            acc = psr_pool.tile([P, F], FP32, tag="acc")
            k = 0
            for j in range(6):  # ring-major = observed fabric service order
                for h in range(2):
                    nc.tensor.matmul(
                        acc[:], identT[:], x_t[j][:, h * F:(h + 1) * F],
                        start=(k == 0), stop=(k == 11),
                        skip_group_check=True,
                    )
                    k += 1

            # ---- logits = acc/12 copied PSUM -> SBUF in halves (DVE+ACT in
            # parallel); this SBUF tile is both the logits output and the
            # solve input (PSUM multi-reader access serializes, SBUF doesn't)
            nc.vector.tensor_scalar_mul(ltT[:, 0:FH], acc[:, 0:FH], 1.0 / H)
            nc.scalar.mul(ltT[:, FH:F], acc[:, FH:F], 1.0 / H)
            nc.gpsimd.dma_start(l_out.ap(), ltT[:])

            # ---- solve rounds
            for r, om in enumerate(OMEGAS):
                nt = nts[r]
                sac = sacs[r]
                # sum r on ACT, into its OWN tile, and EMITTED FIRST: the
                # per-engine semaphore thresholds follow program order, so
                # emitting it after the DVE f-op would stall ACT on it
                rs = scratch.tile([P, F], FP32, name="rs", tag="rs")
                nc.scalar.activation(rs[:], ltT[:], relu_fn, bias=nt[:],
                                     scale=1.0, accum_out=sacRs[r][:])
                fs = scratch.tile([P, F], FP32, name="fs", tag="fs")
                nc.vector._custom_dve(OP_F, out=fs[:], in0=ltT[:], s0=nt[:],
                                      s1=-4.0 / CPR, imm2=1.0,
                                      accum_out=sac[:, 0:1])
                # sum-r hop stays on ACT (Identity copy) — avoids the
                # ACT->DVE->PE double semaphore hop before the reduce
                nc.scalar.add(sac[:, 2:3], sacRs[r][:], 0.0)
                S = psr_pool.tile([P, 3], FP32, name=f"S{r}", tag=f"S{r}")
                nc.tensor.matmul(S[:], wblkT[:], sac[:, 0:3],
                                 start=True, stop=True)
                if r + 1 < nrounds:
                    # lazy n for the NEXT round's quadratic step: sign of
                    # the relu tile on ACT, while the reduce + update run
                    ns_ = scratch.tile([P, F], FP32, name="ns", tag="ns")
                    nc.scalar.activation(ns_[:], rs[:],
                                         mybir.ActivationFunctionType.Sign,
                                         accum_out=sacs[r + 1][:, 1:2])
                # copy the reduced sums to SBUF (a single DVE instruction
                # may read at most one PSUM operand)
                Sb = small.tile([P, 3], FP32, name=f"Sb{r}", tag=f"Sb{r}")
                nc.vector.tensor_scalar_add(Sb[:], S[:], 0.0)
                # update (sr = Sb[:,2] replicated per row; fp = -2*sr folded
                # into the immediates)
                rcs = small.tile([P, 1], FP32, name=f"rcs{r}", tag=f"rcs{r}")
                nc.vector.reciprocal(rcs[:], Sb[:, 2:3])
                if om is not None:
                    # nt1 = nt - (om/2) * f / sr
                    nc.vector._custom_dve(OP_UPDN, out=nts[r + 1][:],
                                          in0=nt[:], in1=Sb[:, 0:1],
                                          s0=rcs[:], imm2=-om / 2.0)
                else:
                    # quadratic step, one sub-Newton:
                    #   e0 = f/(2 sr); q = n*e0 - sr; g = f + e0*(n*e0-2sr)
                    #   nt2 = nt - e0 + 0.5*g/q
                    e0 = small.tile([P, 1], FP32, name=f"e0{r}", tag=f"e0{r}")
                    nc.vector.tensor_scalar(e0[:], Sb[:, 0:1], rcs[:], 0.5,
                                            op0=mult, op1=mult)
                    q = small.tile([P, 1], FP32, name=f"q{r}", tag=f"q{r}")
                    nc.vector.scalar_tensor_tensor(q[:], Sb[:, 1:2], e0[:],
                                                   Sb[:, 2:3], op0=mult,
                                                   op1=subtract)
                    rq = small.tile([P, 1], FP32, name=f"rq{r}", tag=f"rq{r}")
                    nc.vector.reciprocal(rq[:], q[:])
                    g = small.tile([P, 1], FP32, name=f"g{r}", tag=f"g{r}")
                    nc.vector._custom_dve(OP_G, out=g[:], in0=e0[:],
                                          in1=Sb[:, 0:1], s0=Sb[:, 2:3],
                                          s1=Sb[:, 1:2])
                    nc.vector._custom_dve(OP_NTQ, out=nts[r + 1][:],
                                          in0=nt[:], in1=g[:], s0=e0[:],
                                          s1=rq[:], imm2=0.5)

            # ---- final p = sq(relu(L + nt))/4 in halves so the first
            # half's output DMA overlaps the second half's compute
            ntF = nts[nrounds]
            nc.vector._custom_dve(OP_F, out=pT[:, 0:FH], in0=ltT[:, 0:FH],
                                  s0=ntF[:], s1=0.0, imm2=0.25)
            nc.scalar.dma_start(p_out.ap()[:, 0:FH], pT[:, 0:FH])
            nc.vector._custom_dve(OP_F, out=pT[:, FH:F], in0=ltT[:, FH:F],
                                  s0=ntF[:], s1=0.0, imm2=0.25)
            nc.sync.dma_start(p_out.ap()[:, FH:F], pT[:, FH:F])

    nc.compile()
    return nc


_NC = None


def _get_nc():
    global _NC
    if _NC is None:
        _NC = build_nc()
    return _NC


def shard_x(core_slice):
    # [R, H, S] bf16 -> 6 chunk tensors [P, 1024] (2 heads each):
    #   x_j[p, h*512+u] = att[r, 2j+h, c*512+u],  p = r*16 + c
    x = np.ascontiguousarray(
        core_slice.reshape(R, H, CPR, F).transpose(0, 2, 1, 3)  # [r, c, h, u]
    ).reshape(P, H, F)
    return {f"x{j}": np.ascontiguousarray(
        x[:, 2 * j:2 * j + 2, :].reshape(P, 2 * F)) for j in range(6)}


def unshard_out(arr):
    # [P, F] -> [R, S]
    return np.asarray(arr).reshape(R, CPR, F).reshape(R, S)


def _shards(attention):
    att = np.asarray(attention)
    sl = att[:, -1, :, 0, :].astype(BF16_NP)  # [64, 12, 8192] bf16
    return [shard_x(sl[i * R:(i + 1) * R]) for i in range(NCORES)]


def _ensure_ntff_hook():
    """This image's antenv lacks axon_hooks; synthesize it from the boot
    agent's ctypes NTFF driver so trace=True can capture HW profiles."""
    import types

    try:
        from antenv import axon_hooks  # noqa: F401

        return
    except ImportError:
        pass
    import antenv  # noqa: F401
    from trn_agent_boot.trn_boot import _ntff_profile_via_ctypes

    mod = types.ModuleType("antenv.axon_hooks")
    hook = _ntff_profile_via_ctypes("/opt/axon/libaxon_pjrt.so")
    mod.get_axon_ntff_profile_hook = lambda: hook
    mod.set_axon_ntff_profile_hook = lambda h: None
    sys.modules["antenv.axon_hooks"] = mod

    import concourse.bass_utils as bu

    bu.upload_artifacts = lambda tmpdir: tmpdir


def run(attention, trace=False, **trace_kwargs):
    if trace:
        _ensure_ntff_hook()
    nc = _get_nc()
    res = run_bass_kernel_spmd(
        nc,
        _shards(attention),
        core_ids=list(range(NCORES)),
        trace=trace,
        **trace_kwargs,
    )
    p_full = np.concatenate(
        [unshard_out(res.results[i]["p"]) for i in range(NCORES)], axis=0
    )
    l_full = np.concatenate(
        [unshard_out(res.results[i]["logits"]) for i in range(NCORES)], axis=0
    )
    return (p_full, l_full), res


def kernel(attention):
    (p_full, l_full), _ = run(attention, trace=False)
    return p_full, l_full
